# revision 3
# baseline (speedup 1.0000x reference)
"""Multi-head attention (B=2, N=2048, E=1024, H=16) on 8 Trainium2 NeuronCores.

Sharding: data-parallel over batch (2) x tensor-parallel over head-groups (4
groups of 4 heads).  Core c handles batch c//4 and heads 4*(c%4)..4*(c%4)+3.

Host-side shard prep feeds each core feature-major (transposed) activations
and weight shards; the device kernel computes
  qT = Wq_s @ xT + bq_s        (feature-major, [256, 2048])
  kT = Wk_s @ xT + bk_s
  v  = x @ Wv_s.T + bv_s       (position-major, [2048, 256])
  eT[kpos, q] = kT_h.T-contracted energy per head (transposed energy)
  s = exp(eT)                  (no max-subtraction: |logits| < ~60 << 88)
  o  = s.T @ [v | 32]          (ones*32 column yields 32*rowsum in row 64)
  oT normalized by 1/(32*rowsum)  (= softmax / sqrt(E) quirk of the module)
  out_partial = oT.T @ Wp[:, cols].T   (position-major [2048, 1024])
Host sums the 4 head-group partials per batch and adds bp.

All matmuls run in float32r (full-rate fp32 PE mode, ~2e-4 relative error).
"""

import numpy as np

B, N, E, H = 2, 2048, 1024, 16
D = E // H           # 64
NCORES = 8
HG = 4               # head groups
DH = E // HG         # 256 features per head-group
P = 128
NCH = N // 512       # 4 n-chunks of 512
ECH = E // P         # 8 contraction chunks
DCH = DH // P        # 2 feature chunks per shard
KT = N // P          # 16 key tiles
SCALE_COL = float(E ** 0.5)   # 32.0 -> ones column value; row 64 = 32*rowsum

_CACHE = {}


def _build_program():
    import concourse.bacc as bacc
    import concourse.tile as tile
    from concourse import mybir

    F32 = mybir.dt.float32
    F32R = mybir.dt.float32r
    EXP = mybir.ActivationFunctionType.Exp

    nc = bacc.Bacc(None, target_bir_lowering=False, debug=False)

    xqt = nc.declare_dram_parameter("xqt", [E, N], F32R, isOutput=False)
    xkt = nc.declare_dram_parameter("xkt", [E, N], F32R, isOutput=False)
    xvt = nc.declare_dram_parameter("xvt", [E, N], F32R, isOutput=False)
    wqt = nc.declare_dram_parameter("wqt", [E, DH], F32R, isOutput=False)
    wkt = nc.declare_dram_parameter("wkt", [E, DH], F32R, isOutput=False)
    wvt = nc.declare_dram_parameter("wvt", [E, DH], F32R, isOutput=False)
    wpt = nc.declare_dram_parameter("wpt", [DH, E], F32R, isOutput=False)
    bqp = nc.declare_dram_parameter("bq", [DCH, P, 1], F32, isOutput=False)
    bkp = nc.declare_dram_parameter("bk", [DCH, P, 1], F32, isOutput=False)
    bvp = nc.declare_dram_parameter("bv", [1, DH], F32R, isOutput=False)
    onesp = nc.declare_dram_parameter("ones", [1, P], F32R, isOutput=False)
    vonesp = nc.declare_dram_parameter("vones", [P, KT, HG, 1], F32R, isOutput=False)
    out = nc.declare_dram_parameter("out", [N, E], F32, isOutput=True)

    with tile.TileContext(nc) as tc:
        with (
            tc.tile_pool(name="singles", bufs=1) as singles,
            tc.tile_pool(name="xpool", bufs=3) as xpool,
            tc.tile_pool(name="spool", bufs=4) as spool,
            tc.tile_pool(name="npool", bufs=2) as npool,
            tc.tile_pool(name="opool", bufs=2) as opool,
            tc.tile_pool(name="pproj", bufs=1, space="PSUM") as pproj,
            tc.tile_pool(name="peps", bufs=2, space="PSUM") as peps,
            tc.tile_pool(name="ppo", bufs=2, space="PSUM") as ppo,
            tc.tile_pool(name="pbc", bufs=1, space="PSUM") as pbc,
        ):
            # ---- persistent weights / biases ----
            wq_sb = singles.tile([P, ECH, DH], F32R)
            wk_sb = singles.tile([P, ECH, DH], F32R)
            wv_sb = singles.tile([P, ECH, DH], F32R)
            wp_sb = singles.tile([P, DCH, E], F32R)
            nc.sync.dma_start(out=wq_sb, in_=wqt.rearrange("(c p) m -> p c m", p=P))
            nc.sync.dma_start(out=wk_sb, in_=wkt.rearrange("(c p) m -> p c m", p=P))
            nc.sync.dma_start(out=wv_sb, in_=wvt.rearrange("(c p) m -> p c m", p=P))
            nc.sync.dma_start(out=wp_sb, in_=wpt.rearrange("(c p) m -> p c m", p=P))
            bq_sb = singles.tile([P, DCH], F32)
            bk_sb = singles.tile([P, DCH], F32)
            for c in range(DCH):
                nc.sync.dma_start(out=bq_sb[:, c : c + 1], in_=bqp[c])
                nc.sync.dma_start(out=bk_sb[:, c : c + 1], in_=bkp[c])
            bv_sb = singles.tile([1, DH], F32R)
            nc.sync.dma_start(out=bv_sb, in_=bvp[:, :])
            ones1 = singles.tile([1, P], F32R)
            nc.sync.dma_start(out=ones1, in_=onesp[:, :])
            ones1_f = singles.tile([1, P], F32)
            nc.vector.memset(ones1_f, 1.0)

            qT_sb = singles.tile([P, DCH, N], F32R)
            kT_sb = singles.tile([P, DCH, N], F32R)
            oT_sb = singles.tile([P, DCH, N], F32R)
            v_sb = singles.tile([P, KT, HG, D + 1], F32R)
            nc.sync.dma_start(out=v_sb[:, :, :, D : D + 1], in_=vonesp[:, :, :, :])

            # ---- phase 1: projections (streamed over n-chunks of 512) ----
            for ni in range(NCH):
                ns = slice(ni * 512, (ni + 1) * 512)
                # kT
                xk_c = xpool.tile([P, ECH, 512], F32R, tag="x")
                nc.sync.dma_start(
                    out=xk_c, in_=xkt[:, ns].rearrange("(c p) n -> p c n", p=P)
                )
                for dc in range(DCH):
                    ps = pproj.tile([P, 512], F32, tag="proj")
                    for ec in range(ECH):
                        nc.tensor.matmul(
                            ps,
                            wk_sb[:, ec, dc * P : (dc + 1) * P],
                            xk_c[:, ec, :],
                            start=(ec == 0),
                            stop=(ec == ECH - 1),
                        )
                    nc.vector.tensor_scalar_add(
                        kT_sb[:, dc, ns], ps, bk_sb[:, dc : dc + 1]
                    )
                # v (position-major), 4 key-tiles per n-chunk
                xv_c = xpool.tile([P, ECH, 512], F32R, tag="x")
                nc.sync.dma_start(
                    out=xv_c, in_=xvt[:, ns].rearrange("(c p) n -> p c n", p=P)
                )
                for k4 in range(4):
                    kt = ni * 4 + k4
                    vps = pproj.tile([P, DH], F32, tag="proj")
                    nc.tensor.matmul(vps, ones1, bv_sb, start=True, stop=False)
                    for ec in range(ECH):
                        nc.tensor.matmul(
                            vps,
                            xv_c[:, ec, k4 * P : (k4 + 1) * P],
                            wv_sb[:, ec, :],
                            start=False,
                            stop=(ec == ECH - 1),
                        )
                    nc.vector.tensor_copy(
                        v_sb[:, kt, :, 0:D],
                        vps.rearrange("p (h d) -> p h d", h=HG),
                    )
                # qT
                xq_c = xpool.tile([P, ECH, 512], F32R, tag="x")
                nc.sync.dma_start(
                    out=xq_c, in_=xqt[:, ns].rearrange("(c p) n -> p c n", p=P)
                )
                for dc in range(DCH):
                    ps = pproj.tile([P, 512], F32, tag="proj")
                    for ec in range(ECH):
                        nc.tensor.matmul(
                            ps,
                            wq_sb[:, ec, dc * P : (dc + 1) * P],
                            xq_c[:, ec, :],
                            start=(ec == 0),
                            stop=(ec == ECH - 1),
                        )
                    nc.vector.tensor_scalar_add(
                        qT_sb[:, dc, ns], ps, bq_sb[:, dc : dc + 1]
                    )

            # ---- phase 2+3: attention per q-chunk, then output projection ----
            for qc in range(NCH):
                qs = slice(qc * 512, (qc + 1) * 512)
                for pr in range(DCH):  # head pair (2*pr, 2*pr+1)
                    po = [
                        ppo.tile([D + 1, 512], F32, tag="po", name=f"po{hp}")
                        for hp in range(2)
                    ]
                    for ktg in range(KT // 2):
                        eps = [
                            peps.tile([P, 1024], F32, tag="eps", name=f"eps{hp}")
                            for hp in range(2)
                        ]
                        for j in range(2):
                            kt = ktg * 2 + j
                            ks = slice(kt * P, (kt + 1) * P)
                            for hp in range(2):
                                rows = slice(hp * D, (hp + 1) * D)
                                nc.tensor.matmul(
                                    eps[hp][:, j * 512 : (j + 1) * 512],
                                    kT_sb[rows, pr, ks],
                                    qT_sb[rows, pr, qs],
                                    start=True,
                                    stop=True,
                                )
                        sT = [
                            spool.tile([P, 1024], F32R, tag="sT", name=f"sT{hp}")
                            for hp in range(2)
                        ]
                        for hp in range(2):
                            nc.scalar.activation(sT[hp], eps[hp], EXP)
                        for j in range(2):
                            kt = ktg * 2 + j
                            for hp in range(2):
                                nc.tensor.matmul(
                                    po[hp],
                                    v_sb[:, kt, 2 * pr + hp, :],
                                    sT[hp][:, j * 512 : (j + 1) * 512],
                                    start=(kt == 0),
                                    stop=(kt == KT - 1),
                                )
                    # normalize: oT[d, q] = po[d, q] / po[64, q]
                    for hp in range(2):
                        rinv = npool.tile([1, 512], F32, tag="rinv")
                        nc.vector.reciprocal(rinv, po[hp][D : D + 1, :])
                        o_tmp = npool.tile([D, 512], F32, tag="otmp")
                        nc.vector.tensor_copy(o_tmp, po[hp][0:D, :])
                        bc = pbc.tile([D, 512], F32, tag="bc")
                        nc.tensor.matmul(
                            bc, ones1_f[:, 0:D], rinv, start=True, stop=True
                        )
                        nc.vector.tensor_mul(
                            oT_sb[hp * D : (hp + 1) * D, pr, qs], o_tmp, bc
                        )
                # output projection for this q-chunk
                for nt in range(4):
                    n0 = qc * 512 + nt * P
                    osb = opool.tile([P, E], F32, tag="osb")
                    for ecx in range(2):
                        ops = pproj.tile([P, 512], F32, tag="proj")
                        for dc in range(DCH):
                            nc.tensor.matmul(
                                ops,
                                oT_sb[:, dc, n0 : n0 + P],
                                wp_sb[:, dc, ecx * 512 : (ecx + 1) * 512],
                                start=(dc == 0),
                                stop=(dc == DCH - 1),
                            )
                        nc.vector.tensor_copy(osb[:, ecx * 512 : (ecx + 1) * 512], ops)
                    nc.sync.dma_start(out=out[n0 : n0 + P, :], in_=osb)

    nc.compile()
    return nc


def _shard_inputs(queries, keys, values, Wq, bq, Wk, bk, Wv, bv):
    """Host-side shard/layout prep: feature-major activations, transposed
    weight shards.  Returns in_maps for the 8 cores."""
    f32 = np.float32
    xT = {}
    for name, x in (("xqt", queries), ("xkt", keys), ("xvt", values)):
        xT[name] = [np.ascontiguousarray(np.asarray(x[b], f32).T) for b in range(B)]
    maps = []
    for c in range(NCORES):
        b, hg = c // HG, c % HG
        rows = slice(hg * DH, (hg + 1) * DH)
        m = {
            "xqt": xT["xqt"][b],
            "xkt": xT["xkt"][b],
            "xvt": xT["xvt"][b],
            "wqt": np.ascontiguousarray(np.asarray(Wq, f32)[rows].T),
            "wkt": np.ascontiguousarray(np.asarray(Wk, f32)[rows].T),
            "wvt": np.ascontiguousarray(np.asarray(Wv, f32)[rows].T),
            "bq": np.asarray(bq, f32)[rows].reshape(DCH, P, 1).copy(),
            "bk": np.asarray(bk, f32)[rows].reshape(DCH, P, 1).copy(),
            "bv": np.asarray(bv, f32)[rows].reshape(1, DH).copy(),
            "ones": np.ones((1, P), f32),
            "vones": np.full((P, KT, HG, 1), SCALE_COL, f32),
        }
        maps.append(m)
    return maps


def kernel(queries, keys, values, Wq, bq, Wk, bk, Wv, bv, Wp, bp):
    from concourse.bass_utils import run_bass_kernel_spmd

    if "nc" not in _CACHE:
        _CACHE["nc"] = _build_program()
    nc = _CACHE["nc"]

    in_maps = _shard_inputs(queries, keys, values, Wq, bq, Wk, bk, Wv, bv)
    Wp = np.asarray(Wp, np.float32)
    for c in range(NCORES):
        hg = c % HG
        rows = slice(hg * DH, (hg + 1) * DH)
        in_maps[c]["wpt"] = np.ascontiguousarray(Wp[:, rows].T)

    res = run_bass_kernel_spmd(nc, in_maps, list(range(NCORES)))

    out = np.zeros((B, N, E), np.float32)
    for c in range(NCORES):
        out[c // HG] += res.results[c]["out"]
    out += np.asarray(bp, np.float32)
    return out
